# revision 19
# baseline (speedup 1.0000x reference)
"""Trainium2 Bass kernel for nn_Decoder (mean-pool L=16 + overlap-add step 8).

Math (per (b, c) slice, est = est_source[b, c] of shape [256, 4000]):
  A[g, f]      = (1/16) * sum_{l=0..15} est[16*g + l, f]          g in 0..15
  out[8*s + j] = A[j, s] + A[8+j, s-1]                            s in 0..4000
with A[., -1] = A[., 4000] = 0 at the edges.  Output length 8*4001 = 32008.

Kernel strategy (8 cores, 4 slices each).  The group-of-16 partition
reduction is a matmul with a block 1/16 weight matrix W [128, 8]; frame axis
is the matmul output partition dim (lhsT = z tile [128 d, 128 s], rhs = W)
so PSUM tiles come out [128 s, 8 j], matching the interleaved output layout.
The overlap-add is folded into one DVE add per chunk: the host packs, for
each 2048-frame chunk, low-half rows and (+1-frame-shifted) high-half rows
adjacently, so each chunk is ONE contiguous [128, 8 KiB] DMA (single
completion semaphore -> the DVE add carries exactly one sync wait, as the
pinned walrus build requires; 8 KiB-contiguous DRAM rows give max-size DMA
packets).

Pipeline layout (per core: 8 chunks of 2048 frames):
  sync ring   : all 8 load triggers issued up-front (FIFO completion,
                one chunk lands every ~2.9 us at ~352 GB/s)
  vector      : z = xl + xh   (one 2x-mode fp16 add per chunk)
  tensor      : 16 (ldweights+matmul) per chunk into one PSUM bank;
                a warmup matmul absorbs the W-load DMA wait so no real
                matmul carries two waits; it targets chunk 0's bank so
                8 chunks + warmup fit the 8 PSUM banks
  scalar      : W load trigger, then per chunk: PSUM->SBUF copy + store
                trigger on the second HWDGE ring (qActDynamicHW), disjoint
                from the load ring.  Stores are [128 x 512B] contiguous
                runs; the host transposes the output instead (layout-only).

Everything is fully resident in SBUF (no buffer reuse -> no extra waits).
"""

import sys

if "/opt/trn_rl_repo" not in sys.path:
    sys.path.insert(0, "/opt/trn_rl_repo")

import numpy as np


def _install_ntff_hook():
    """Provide antenv.axon_hooks (absent in this image) so trace=True works.

    The boot-side installer (trn_agent_boot.trn_boot) skips hook setup when
    antenv.axon_hooks is missing; bass_utils then refuses to trace.  We
    register a lazy equivalent backed by the same ctypes NTFF driver.
    """
    import types
    try:
        import antenv
    except ImportError:
        return
    if "antenv.axon_hooks" in sys.modules:
        return
    mod = types.ModuleType("antenv.axon_hooks")
    _state = {}

    def set_axon_ntff_profile_hook(h):
        _state["h"] = h

    def get_axon_ntff_profile_hook():
        if "h" not in _state:
            try:
                from trn_agent_boot.trn_boot import _ntff_profile_via_ctypes
                _state["h"] = _ntff_profile_via_ctypes("/opt/axon/libaxon_pjrt.so")
            except Exception:
                _state["h"] = None
        return _state["h"]

    mod.set_axon_ntff_profile_hook = set_axon_ntff_profile_hook
    mod.get_axon_ntff_profile_hook = get_axon_ntff_profile_hook
    sys.modules["antenv.axon_hooks"] = mod
    antenv.axon_hooks = mod


_install_ntff_hook()

import concourse.bass as bass
import concourse.mybir as mybir
from concourse import tile
from concourse.bass_utils import run_bass_kernel_spmd


class _SingleWaitTileContext(tile.TileContext):
    """TileContext whose kernel-tail drain never carries multiple sem waits.

    The pinned walrus build rejects any instruction with more than one sync
    wait ("Too many sync wait commands").  Tile's default exit emits a single
    Drain waiting on every outstanding proc semaphore.  Instead, emit one
    wait_ge per proc on the SP sequencer (each a single-wait instruction),
    then a wait-free drain.
    """

    # proc indices >= _FIRST_DMA_PROC are DMA lanes whose semaphores advance
    # by 16 per op (one inc per SDMA engine) while the vector clock ticks 1.
    _FIRST_DMA_PROC = 11

    def _drain_and_barrier(self, tick_clock, wait_clock):
        nc = self.nc
        clock = tick_clock.global_clock  # bass_rust.VectorClock: 27 ints
        allocated = wait_clock.sems.allocated()
        for proc_idx, tick in enumerate(clock):
            if tick > 0 and proc_idx in allocated:
                val = tick * 16 if proc_idx >= self._FIRST_DMA_PROC else tick
                nc.sync.wait_ge(allocated[proc_idx], val)
        nc.sync.drain()
        nc.all_engine_barrier()
        popped = nc._tile_sem_poison_stack.pop()
        assert popped is self._sem_poison
        nc.clear_and_free_semaphores(list(self.sems.allocated().values()))
        nc.all_engine_barrier()

# Problem constants (hardcoded per spec)
B, C, D2, FRAMES = 16, 2, 256, 4000
L = 16
SUB = FRAMES + 1          # 4001 output subframes per slice
OUT_LEN = 8 * SUB         # 32008
N_CORES = 8
SLICES = (B * C) // N_CORES   # 4 slices per core
FTILE = 128               # frames per matmul tile
CH = 2048                 # frames per pipeline chunk
PADF = 4096               # padded frames per slice (2 chunks)
NCH = PADF // CH          # chunks per slice
TPC = CH // FTILE         # matmul tiles per chunk (16)
NCHUNK = SLICES * NCH     # chunks per core (8)
# Final chunk's sub-splits (frames): short end-of-pipeline chain.
SUBS = [1024, 512, 512]

MM_DT_NP = np.float16     # matmul operand dtype (~1e-4 rel err, halves HBM)

_CACHE = {}


def _build_w() -> np.ndarray:
    w = np.zeros((128, 8), dtype=np.float32)
    for j in range(8):
        w[16 * j : 16 * j + 16, j] = 1.0 / L
    return w


def _build_nc() -> bass.Bass:
    mm_dt = mybir.dt.float16
    nc = bass.Bass()
    # Host-packed input: xz[i, c, d, 0:CH]    = low-half rows of chunk c;
    #                    xz[i, c, d, CH:2CH]  = high-half rows shifted +1.
    # Slice-major rows: xz[i, d] = [chunk0: xl|xh | chunk1: xl|xh], 16 KiB
    # contiguous per row -> max-size DMA packets for the merged loads.
    xz_d = nc.dram_tensor("xz", [SLICES, 128, 2 * PADF], mm_dt,
                          kind="ExternalInput")
    w = nc.dram_tensor("w", [128, 8], mm_dt, kind="ExternalInput")
    # Output, partition-major: y[i, p, 128*c + 8*t_local + j]; the host
    # transposes to frame-major ((t*128+p)*8 + j), converts to fp32, and
    # trims to OUT_LEN.  fp16 output halves store traffic (values are
    # matmul fp32 results; the fp16 round adds ~5e-4 rel err, well within
    # tolerance).
    y = nc.dram_tensor("y", [SLICES, 128, NCH * TPC * 8], mm_dt,
                       kind="ExternalOutput")

    with _SingleWaitTileContext(nc) as tc:
        with (
            tc.tile_pool(name="wp", bufs=1) as wp,
            tc.tile_pool(name="xz", bufs=NCHUNK) as xzp,
            tc.tile_pool(name="zs", bufs=NCHUNK + len(SUBS) - 1) as zsp,
            tc.tile_pool(name="ob", bufs=NCHUNK) as obp,
            tc.tile_pool(name="ps", bufs=8, space="PSUM") as psp,
        ):
            # W load on the scalar (qActDynamicHW) ring; chunk loads own the
            # sync (qSPDynamicHW) ring so they complete FIFO at full rate.
            wt = wp.tile([128, 8], mm_dt)
            nc.scalar.dma_start(out=wt[:], in_=w[:])

            # All load triggers up-front on the sync ring.  Early slices load
            # as merged 2 MiB transfers (fewer ring handoffs); the final
            # chunk is split [1024, 512, 512] so the end-of-kernel chain
            # (add -> matmuls -> copy -> store) runs on 512 frames.  7 load
            # triggers + W = 8 HWDGE DMAs = the 8 DMAHW sem lanes exactly,
            # so no DMA needs a lane-reuse wait (walrus: 1 sync wait/inst).
            xt = {}          # chunk k -> (tile, column offset of chunk data)
            for i in range(3):   # slices 0-2: one 2 MiB load for both chunks
                t = xzp.tile([128, 2 * PADF], mm_dt, tag="xpair",
                             name=f"xp{i}", bufs=3)
                nc.sync.dma_start(out=t[:], in_=xz_d[i])
                xt[2 * i] = (t, 0)
                xt[2 * i + 1] = (t, 2 * CH)
            x6 = xzp.tile([128, 2 * CH], mm_dt, tag="xone", name="x6", bufs=2)
            nc.sync.dma_start(out=x6[:], in_=xz_d[3][:, 0 : 2 * CH])
            xt[6] = (x6, 0)
            x7 = xzp.tile([128, 2 * CH], mm_dt, tag="xone", name="x7", bufs=2)
            off = 0
            for n in SUBS:
                nc.sync.dma_start(
                    out=x7[:, off : off + 2 * n],
                    in_=xz_d[3][:, 2 * CH + off : 2 * CH + off + 2 * n])
                off += 2 * n
            xt[7] = (x7, 0)

            # One full PSUM bank per chunk (8 chunks = 8 banks); full-bank
            # tiles guarantee no PE-write/ScalarE-read bank sharing.
            ps = [psp.tile([128, 512], mybir.dt.float32, tag="ps",
                           name=f"ps{k}")
                  for k in range(NCHUNK)]

            # Warmup matmul: absorbs the W-load DMA wait so no real matmul
            # ever carries two sync waits (walrus limit).  Writes garbage
            # into chunk 0's bank; chunk 0's first matmul overwrites it.
            nc.tensor.matmul(ps[0][0:8, 0:8], wt[:], wt[:],
                             start=True, stop=True)

            for k in range(NCHUNK):
                subs = [CH] if k < NCHUNK - 1 else SUBS
                base, _ = xt[k]
                xoff = xt[k][1]
                t0 = 0
                for n in subs:
                    z = zsp.tile([128, CH], mm_dt)
                    nc.vector.tensor_tensor(
                        out=z[:, 0:n],
                        in0=base[:, xoff : xoff + n],
                        in1=base[:, xoff + n : xoff + 2 * n],
                        op=mybir.AluOpType.add)
                    for t in range(n // FTILE):
                        nc.tensor.matmul(
                            ps[k][:, 8 * (t0 + t) : 8 * (t0 + t) + 8],
                            z[:, FTILE * t : FTILE * (t + 1)],
                            wt[:],
                            start=True, stop=True,
                        )
                    t0 += n // FTILE
                    xoff += 2 * n
                ob = obp.tile([128, 8 * TPC], mm_dt)
                nc.scalar.copy(ob[:], ps[k][:, 0 : 8 * TPC])
                i, c = k // NCH, k % NCH
                # SWDGE store: keeps the 8 DMASW sem lanes for stores and the
                # 8 DMAHW lanes for loads, so no DMA ever needs a lane-reuse
                # wait on top of its data wait (walrus: 1 sync wait/inst).
                nc.gpsimd.dma_start(
                    out=y[i][:, 8 * TPC * c : 8 * TPC * (c + 1)],
                    in_=ob[:],
                )
    return nc


def _get_nc():
    if "nc" not in _CACHE:
        _CACHE["nc"] = _build_nc()
    return _CACHE["nc"]


def _pack_span(xz, e16, c, col0, lo, n):
    """Pack frames [lo, lo+n) as [xl(n) | xh(n)] at chunk c, column col0."""
    hi = min(FRAMES, lo + n)
    if hi > lo:
        xz[:, c, :, col0 : col0 + hi - lo] = e16[:, 0:128, lo:hi]
    ls = max(1, lo)                       # first valid global s in the span
    le = min(lo + n - 1, FRAMES)          # last valid global s
    if le >= ls:
        xz[:, c, :, col0 + n + (ls - lo) : col0 + n + (le - lo) + 1] = \
            e16[:, 128:256, ls - 1 : le]


def _prep_inputs(est: np.ndarray) -> np.ndarray:
    """Pack [S, 256, F] fp32 into per-chunk [S, NCH, 128, 2*CH] fp16 tiles.

    Chunk layout per d-row: [xl(CH) | xh(CH)] where xl holds low rows for the
    chunk's frames (zero-padded) and xh the high rows shifted +1 frame.  The
    final chunk (c = NCH-1) of the final slice in each core's group of
    SLICES is sub-split [xlA(CHA)|xhA(CHA)|xlB(CH-CHA)|xhB(CH-CHA)] to match
    the kernel's short end-of-pipeline chain.
    """
    S = est.shape[0]
    e16 = est.astype(MM_DT_NP)
    xz = np.zeros((S, NCH, 128, 2 * CH), dtype=MM_DT_NP)
    for c in range(NCH):
        _pack_span(xz, e16, c, 0, c * CH, CH)
    # re-pack the split chunk for the last slice of each core group
    split = np.zeros((S, 128, 2 * CH), dtype=MM_DT_NP)
    sv = split[:, None]  # [S, 1, 128, 2CH] view so _pack_span's c=0 indexes it
    col = 0
    lo = (NCH - 1) * CH
    for n in SUBS:
        _pack_span(sv, e16, 0, col, lo, n)
        col += 2 * n
        lo += n
    last = np.arange(S) % SLICES == SLICES - 1
    xz[last, NCH - 1] = split[last]
    # [S, NCH, 128, 2CH] -> slice-major rows [S, 128, NCH*2CH]
    return np.ascontiguousarray(xz.transpose(0, 2, 1, 3)).reshape(
        S, 128, 2 * PADF)


def kernel(est_source: np.ndarray, _trace: bool = False) -> np.ndarray:
    est = np.ascontiguousarray(np.asarray(est_source), dtype=np.float32)
    assert est.shape == (B, C, D2, FRAMES)
    flat = est.reshape(B * C, D2, FRAMES)
    xz = _prep_inputs(flat)
    wmat = _build_w().astype(MM_DT_NP)

    nc = _get_nc()
    in_maps = [
        {"xz": xz[SLICES * k : SLICES * (k + 1)], "w": wmat}
        for k in range(N_CORES)
    ]
    res = run_bass_kernel_spmd(nc, in_maps, core_ids=list(range(N_CORES)),
                               trace=_trace)
    _CACHE["last_results"] = res
    # y[i, p, 128c + 8t_local + j] -> out[i, (128(16c+t_local)+p)*8 + j]
    ys = [res.results[k]["y"] for k in range(N_CORES)]
    y = np.concatenate(ys, axis=0).astype(np.float32)    # [32, 128, 256]
    y = y.reshape(B * C, 128, NCH * TPC, 8)              # [32, p, t, j]
    y = y.transpose(0, 2, 1, 3).reshape(B * C, PADF * 8)  # [(t p j)]
    return np.ascontiguousarray(y[:, :OUT_LEN]).reshape(B, C, OUT_LEN)


# revision 20
# speedup vs baseline: 1.0368x; 1.0368x over previous
"""Trainium2 Bass kernel for nn_Decoder (mean-pool L=16 + overlap-add step 8).

Math (per (b, c) slice, est = est_source[b, c] of shape [256, 4000]):
  A[g, f]      = (1/16) * sum_{l=0..15} est[16*g + l, f]          g in 0..15
  out[8*s + j] = A[j, s] + A[8+j, s-1]                            s in 0..4000
with A[., -1] = A[., 4000] = 0 at the edges.  Output length 8*4001 = 32008.

Kernel strategy (8 cores, 4 slices each).  The group-of-16 partition
reduction is a matmul with a block 1/16 weight matrix W [128, 8]; frame axis
is the matmul output partition dim (lhsT = z tile [128 d, 128 s], rhs = W)
so PSUM tiles come out [128 s, 8 j], matching the interleaved output layout.
The overlap-add is folded into one DVE add per chunk: the host packs, for
each 2048-frame chunk, low-half rows and (+1-frame-shifted) high-half rows
adjacently, so each chunk is ONE contiguous [128, 8 KiB] DMA (single
completion semaphore -> the DVE add carries exactly one sync wait, as the
pinned walrus build requires; 8 KiB-contiguous DRAM rows give max-size DMA
packets).

Pipeline layout (per core: 8 chunks of 2048 frames):
  sync ring   : all 8 load triggers issued up-front (FIFO completion,
                one chunk lands every ~2.9 us at ~352 GB/s)
  vector      : z = xl + xh   (one 2x-mode fp16 add per chunk)
  tensor      : 16 (ldweights+matmul) per chunk into one PSUM bank;
                a warmup matmul absorbs the W-load DMA wait so no real
                matmul carries two waits; it targets chunk 0's bank so
                8 chunks + warmup fit the 8 PSUM banks
  scalar      : W load trigger, then per chunk: PSUM->SBUF copy + store
                trigger on the second HWDGE ring (qActDynamicHW), disjoint
                from the load ring.  Stores are [128 x 512B] contiguous
                runs; the host transposes the output instead (layout-only).

Everything is fully resident in SBUF (no buffer reuse -> no extra waits).
"""

import sys

if "/opt/trn_rl_repo" not in sys.path:
    sys.path.insert(0, "/opt/trn_rl_repo")

import numpy as np


def _install_ntff_hook():
    """Provide antenv.axon_hooks (absent in this image) so trace=True works.

    The boot-side installer (trn_agent_boot.trn_boot) skips hook setup when
    antenv.axon_hooks is missing; bass_utils then refuses to trace.  We
    register a lazy equivalent backed by the same ctypes NTFF driver.
    """
    import types
    try:
        import antenv
    except ImportError:
        return
    if "antenv.axon_hooks" in sys.modules:
        return
    mod = types.ModuleType("antenv.axon_hooks")
    _state = {}

    def set_axon_ntff_profile_hook(h):
        _state["h"] = h

    def get_axon_ntff_profile_hook():
        if "h" not in _state:
            try:
                from trn_agent_boot.trn_boot import _ntff_profile_via_ctypes
                _state["h"] = _ntff_profile_via_ctypes("/opt/axon/libaxon_pjrt.so")
            except Exception:
                _state["h"] = None
        return _state["h"]

    mod.set_axon_ntff_profile_hook = set_axon_ntff_profile_hook
    mod.get_axon_ntff_profile_hook = get_axon_ntff_profile_hook
    sys.modules["antenv.axon_hooks"] = mod
    antenv.axon_hooks = mod


_install_ntff_hook()

import concourse.bass as bass
import concourse.mybir as mybir
from concourse import tile
from concourse.bass_utils import run_bass_kernel_spmd


class _SingleWaitTileContext(tile.TileContext):
    """TileContext whose kernel-tail drain never carries multiple sem waits.

    The pinned walrus build rejects any instruction with more than one sync
    wait ("Too many sync wait commands").  Tile's default exit emits a single
    Drain waiting on every outstanding proc semaphore.  Instead, emit one
    wait_ge per proc on the SP sequencer (each a single-wait instruction),
    then a wait-free drain.
    """

    # proc indices >= _FIRST_DMA_PROC are DMA lanes whose semaphores advance
    # by 16 per op (one inc per SDMA engine) while the vector clock ticks 1.
    _FIRST_DMA_PROC = 11

    def _drain_and_barrier(self, tick_clock, wait_clock):
        nc = self.nc
        clock = tick_clock.global_clock  # bass_rust.VectorClock: 27 ints
        allocated = wait_clock.sems.allocated()
        for proc_idx, tick in enumerate(clock):
            if tick > 0 and proc_idx in allocated:
                val = tick * 16 if proc_idx >= self._FIRST_DMA_PROC else tick
                nc.sync.wait_ge(allocated[proc_idx], val)
        nc.sync.drain()
        nc.all_engine_barrier()
        popped = nc._tile_sem_poison_stack.pop()
        assert popped is self._sem_poison
        nc.clear_and_free_semaphores(list(self.sems.allocated().values()))
        nc.all_engine_barrier()

# Problem constants (hardcoded per spec)
B, C, D2, FRAMES = 16, 2, 256, 4000
L = 16
SUB = FRAMES + 1          # 4001 output subframes per slice
OUT_LEN = 8 * SUB         # 32008
N_CORES = 8
SLICES = (B * C) // N_CORES   # 4 slices per core
FTILE = 128               # frames per matmul tile
CH = 2048                 # frames per pipeline chunk
PADF = 4096               # padded frames per slice (2 chunks)
NCH = PADF // CH          # chunks per slice
TPC = CH // FTILE         # matmul tiles per chunk (16)
NCHUNK = SLICES * NCH     # chunks per core (8)
# Final chunk's sub-splits (frames): short end-of-pipeline chain.
SUBS = [1024, 512, 512]

MM_DT_NP = np.float16     # matmul operand dtype (~1e-4 rel err, halves HBM)

_CACHE = {}


def _build_w() -> np.ndarray:
    w = np.zeros((128, 8), dtype=np.float32)
    for j in range(8):
        w[16 * j : 16 * j + 16, j] = 1.0 / L
    return w


def _build_nc() -> bass.Bass:
    mm_dt = mybir.dt.float16
    nc = bass.Bass()
    # Host-packed input: xz[i, c, d, 0:CH]    = low-half rows of chunk c;
    #                    xz[i, c, d, CH:2CH]  = high-half rows shifted +1.
    # Slice-major rows: xz[i, d] = [chunk0: xl|xh | chunk1: xl|xh], 16 KiB
    # contiguous per row -> max-size DMA packets for the merged loads.
    xz_d = nc.dram_tensor("xz", [SLICES, 128, 2 * PADF], mm_dt,
                          kind="ExternalInput")
    w = nc.dram_tensor("w", [128, 8], mm_dt, kind="ExternalInput")
    # Output, partition-major: y[i, p, 128*c + 8*t_local + j]; the host
    # transposes to frame-major ((t*128+p)*8 + j), converts to fp32, and
    # trims to OUT_LEN.  fp16 output halves store traffic (values are
    # matmul fp32 results; the fp16 round adds ~5e-4 rel err, well within
    # tolerance).
    y = nc.dram_tensor("y", [SLICES, 128, NCH * TPC * 8], mm_dt,
                       kind="ExternalOutput")

    with _SingleWaitTileContext(nc) as tc:
        with (
            tc.tile_pool(name="wp", bufs=1) as wp,
            tc.tile_pool(name="xz", bufs=NCHUNK) as xzp,
            tc.tile_pool(name="zs", bufs=NCHUNK + len(SUBS) - 1) as zsp,
            tc.tile_pool(name="ob", bufs=NCHUNK) as obp,
            tc.tile_pool(name="ps", bufs=8, space="PSUM") as psp,
        ):
            # W load on the scalar (qActDynamicHW) ring; chunk loads own the
            # sync (qSPDynamicHW) ring so they complete FIFO at full rate.
            wt = wp.tile([128, 8], mm_dt)
            nc.scalar.dma_start(out=wt[:], in_=w[:])

            # All load triggers up-front on the sync ring.  Early slices load
            # as merged 2 MiB transfers (fewer ring handoffs); the final
            # chunk is split [1024, 512, 512] so the end-of-kernel chain
            # (add -> matmuls -> copy -> store) runs on 512 frames.  7 load
            # triggers + W = 8 HWDGE DMAs = the 8 DMAHW sem lanes exactly,
            # so no DMA needs a lane-reuse wait (walrus: 1 sync wait/inst).
            xt = {}          # chunk k -> (tile, column offset of chunk data)
            for k in range(NCHUNK - 1):
                t = xzp.tile([128, 2 * CH], mm_dt, tag="xc", name=f"xc{k}",
                             bufs=NCHUNK)
                i, c = k // NCH, k % NCH
                nc.sync.dma_start(
                    out=t[:], in_=xz_d[i][:, 2 * CH * c : 2 * CH * (c + 1)])
                xt[k] = (t, 0)
            x7 = xzp.tile([128, 2 * CH], mm_dt, tag="xc", name="x7",
                          bufs=NCHUNK)
            off = 0
            for n in SUBS:
                nc.sync.dma_start(
                    out=x7[:, off : off + 2 * n],
                    in_=xz_d[3][:, 2 * CH + off : 2 * CH + off + 2 * n])
                off += 2 * n
            xt[NCHUNK - 1] = (x7, 0)

            # One full PSUM bank per chunk (8 chunks = 8 banks); full-bank
            # tiles guarantee no PE-write/ScalarE-read bank sharing.
            ps = [psp.tile([128, 512], mybir.dt.float32, tag="ps",
                           name=f"ps{k}")
                  for k in range(NCHUNK)]

            # Warmup matmul: absorbs the W-load DMA wait so no real matmul
            # ever carries two sync waits (walrus limit).  Writes garbage
            # into chunk 0's bank; chunk 0's first matmul overwrites it.
            nc.tensor.matmul(ps[0][0:8, 0:8], wt[:], wt[:],
                             start=True, stop=True)

            for k in range(NCHUNK):
                subs = [CH] if k < NCHUNK - 1 else SUBS
                base, _ = xt[k]
                xoff = xt[k][1]
                t0 = 0
                for n in subs:
                    z = zsp.tile([128, CH], mm_dt)
                    nc.vector.tensor_tensor(
                        out=z[:, 0:n],
                        in0=base[:, xoff : xoff + n],
                        in1=base[:, xoff + n : xoff + 2 * n],
                        op=mybir.AluOpType.add)
                    for t in range(n // FTILE):
                        nc.tensor.matmul(
                            ps[k][:, 8 * (t0 + t) : 8 * (t0 + t) + 8],
                            z[:, FTILE * t : FTILE * (t + 1)],
                            wt[:],
                            start=True, stop=True,
                        )
                    t0 += n // FTILE
                    xoff += 2 * n
                ob = obp.tile([128, 8 * TPC], mm_dt)
                nc.scalar.copy(ob[:], ps[k][:, 0 : 8 * TPC])
                i, c = k // NCH, k % NCH
                # SWDGE store: keeps the 8 DMASW sem lanes for stores and the
                # 8 DMAHW lanes for loads, so no DMA ever needs a lane-reuse
                # wait on top of its data wait (walrus: 1 sync wait/inst).
                nc.gpsimd.dma_start(
                    out=y[i][:, 8 * TPC * c : 8 * TPC * (c + 1)],
                    in_=ob[:],
                )
    return nc


def _get_nc():
    if "nc" not in _CACHE:
        _CACHE["nc"] = _build_nc()
    return _CACHE["nc"]


def _pack_span(xz, e16, c, col0, lo, n):
    """Pack frames [lo, lo+n) as [xl(n) | xh(n)] at chunk c, column col0."""
    hi = min(FRAMES, lo + n)
    if hi > lo:
        xz[:, c, :, col0 : col0 + hi - lo] = e16[:, 0:128, lo:hi]
    ls = max(1, lo)                       # first valid global s in the span
    le = min(lo + n - 1, FRAMES)          # last valid global s
    if le >= ls:
        xz[:, c, :, col0 + n + (ls - lo) : col0 + n + (le - lo) + 1] = \
            e16[:, 128:256, ls - 1 : le]


def _prep_inputs(est: np.ndarray) -> np.ndarray:
    """Pack [S, 256, F] fp32 into per-chunk [S, NCH, 128, 2*CH] fp16 tiles.

    Chunk layout per d-row: [xl(CH) | xh(CH)] where xl holds low rows for the
    chunk's frames (zero-padded) and xh the high rows shifted +1 frame.  The
    final chunk (c = NCH-1) of the final slice in each core's group of
    SLICES is sub-split [xlA(CHA)|xhA(CHA)|xlB(CH-CHA)|xhB(CH-CHA)] to match
    the kernel's short end-of-pipeline chain.
    """
    S = est.shape[0]
    e16 = est.astype(MM_DT_NP)
    xz = np.zeros((S, NCH, 128, 2 * CH), dtype=MM_DT_NP)
    for c in range(NCH):
        _pack_span(xz, e16, c, 0, c * CH, CH)
    # re-pack the split chunk for the last slice of each core group
    split = np.zeros((S, 128, 2 * CH), dtype=MM_DT_NP)
    sv = split[:, None]  # [S, 1, 128, 2CH] view so _pack_span's c=0 indexes it
    col = 0
    lo = (NCH - 1) * CH
    for n in SUBS:
        _pack_span(sv, e16, 0, col, lo, n)
        col += 2 * n
        lo += n
    last = np.arange(S) % SLICES == SLICES - 1
    xz[last, NCH - 1] = split[last]
    # [S, NCH, 128, 2CH] -> slice-major rows [S, 128, NCH*2CH]
    return np.ascontiguousarray(xz.transpose(0, 2, 1, 3)).reshape(
        S, 128, 2 * PADF)


def kernel(est_source: np.ndarray, _trace: bool = False) -> np.ndarray:
    est = np.ascontiguousarray(np.asarray(est_source), dtype=np.float32)
    assert est.shape == (B, C, D2, FRAMES)
    flat = est.reshape(B * C, D2, FRAMES)
    xz = _prep_inputs(flat)
    wmat = _build_w().astype(MM_DT_NP)

    nc = _get_nc()
    in_maps = [
        {"xz": xz[SLICES * k : SLICES * (k + 1)], "w": wmat}
        for k in range(N_CORES)
    ]
    res = run_bass_kernel_spmd(nc, in_maps, core_ids=list(range(N_CORES)),
                               trace=_trace)
    _CACHE["last_results"] = res
    # y[i, p, 128c + 8t_local + j] -> out[i, (128(16c+t_local)+p)*8 + j]
    ys = [res.results[k]["y"] for k in range(N_CORES)]
    y = np.concatenate(ys, axis=0).astype(np.float32)    # [32, 128, 256]
    y = y.reshape(B * C, 128, NCH * TPC, 8)              # [32, p, t, j]
    y = y.transpose(0, 2, 1, 3).reshape(B * C, PADF * 8)  # [(t p j)]
    return np.ascontiguousarray(y[:, :OUT_LEN]).reshape(B, C, OUT_LEN)


# revision 21
# speedup vs baseline: 1.1237x; 1.0838x over previous
"""Trainium2 Bass kernel for nn_Decoder (mean-pool L=16 + overlap-add step 8).

Math (per (b, c) slice, est = est_source[b, c] of shape [256, 4000]):
  A[g, f]      = (1/16) * sum_{l=0..15} est[16*g + l, f]          g in 0..15
  out[8*s + j] = A[j, s] + A[8+j, s-1]                            s in 0..4000
with A[., -1] = A[., 4000] = 0 at the edges.  Output length 8*4001 = 32008.

Kernel strategy (8 cores, 4 slices each).  The group-of-16 partition
reduction is a matmul with a block 1/16 weight matrix W [128, 8]; frame axis
is the matmul output partition dim (lhsT = z tile [128 d, 128 s], rhs = W)
so PSUM tiles come out [128 s, 8 j], matching the interleaved output layout.
The overlap-add is folded into one DVE add per chunk: the host packs, for
each 2048-frame chunk, low-half rows and (+1-frame-shifted) high-half rows
adjacently, so each chunk is ONE contiguous [128, 8 KiB] DMA (single
completion semaphore -> the DVE add carries exactly one sync wait, as the
pinned walrus build requires; 8 KiB-contiguous DRAM rows give max-size DMA
packets).

Pipeline layout (per core: 8 chunks of 2048 frames):
  sync ring   : all 8 load triggers issued up-front (FIFO completion,
                one chunk lands every ~2.9 us at ~352 GB/s)
  vector      : z = xl + xh   (one 2x-mode fp16 add per chunk)
  tensor      : 16 (ldweights+matmul) per chunk into one PSUM bank;
                a warmup matmul absorbs the W-load DMA wait so no real
                matmul carries two waits; it targets chunk 0's bank so
                8 chunks + warmup fit the 8 PSUM banks
  scalar      : W load trigger, then per chunk: PSUM->SBUF copy + store
                trigger on the second HWDGE ring (qActDynamicHW), disjoint
                from the load ring.  Stores are [128 x 512B] contiguous
                runs; the host transposes the output instead (layout-only).

Everything is fully resident in SBUF (no buffer reuse -> no extra waits).
"""

import sys

if "/opt/trn_rl_repo" not in sys.path:
    sys.path.insert(0, "/opt/trn_rl_repo")

import numpy as np


def _install_ntff_hook():
    """Provide antenv.axon_hooks (absent in this image) so trace=True works.

    The boot-side installer (trn_agent_boot.trn_boot) skips hook setup when
    antenv.axon_hooks is missing; bass_utils then refuses to trace.  We
    register a lazy equivalent backed by the same ctypes NTFF driver.
    """
    import types
    try:
        import antenv
    except ImportError:
        return
    if "antenv.axon_hooks" in sys.modules:
        return
    mod = types.ModuleType("antenv.axon_hooks")
    _state = {}

    def set_axon_ntff_profile_hook(h):
        _state["h"] = h

    def get_axon_ntff_profile_hook():
        if "h" not in _state:
            try:
                from trn_agent_boot.trn_boot import _ntff_profile_via_ctypes
                _state["h"] = _ntff_profile_via_ctypes("/opt/axon/libaxon_pjrt.so")
            except Exception:
                _state["h"] = None
        return _state["h"]

    mod.set_axon_ntff_profile_hook = set_axon_ntff_profile_hook
    mod.get_axon_ntff_profile_hook = get_axon_ntff_profile_hook
    sys.modules["antenv.axon_hooks"] = mod
    antenv.axon_hooks = mod


_install_ntff_hook()

import concourse.bass as bass
import concourse.mybir as mybir
from concourse import tile
from concourse.bass_utils import run_bass_kernel_spmd


class _SingleWaitTileContext(tile.TileContext):
    """TileContext whose kernel-tail drain never carries multiple sem waits.

    The pinned walrus build rejects any instruction with more than one sync
    wait ("Too many sync wait commands").  Tile's default exit emits a single
    Drain waiting on every outstanding proc semaphore.  Instead, emit one
    wait_ge per proc on the SP sequencer (each a single-wait instruction),
    then a wait-free drain.
    """

    # proc indices >= _FIRST_DMA_PROC are DMA lanes whose semaphores advance
    # by 16 per op (one inc per SDMA engine) while the vector clock ticks 1.
    _FIRST_DMA_PROC = 11

    def _drain_and_barrier(self, tick_clock, wait_clock):
        nc = self.nc
        clock = tick_clock.global_clock  # bass_rust.VectorClock: 27 ints
        allocated = wait_clock.sems.allocated()
        for proc_idx, tick in enumerate(clock):
            if tick > 0 and proc_idx in allocated:
                val = tick * 16 if proc_idx >= self._FIRST_DMA_PROC else tick
                nc.sync.wait_ge(allocated[proc_idx], val)
        nc.sync.drain()
        nc.all_engine_barrier()
        popped = nc._tile_sem_poison_stack.pop()
        assert popped is self._sem_poison
        nc.clear_and_free_semaphores(list(self.sems.allocated().values()))
        nc.all_engine_barrier()

# Problem constants (hardcoded per spec)
B, C, D2, FRAMES = 16, 2, 256, 4000
L = 16
SUB = FRAMES + 1          # 4001 output subframes per slice
OUT_LEN = 8 * SUB         # 32008
N_CORES = 8
SLICES = (B * C) // N_CORES   # 4 slices per core
FTILE = 128               # frames per matmul tile
CH = 2048                 # frames per pipeline chunk
PADF = 4096               # padded frames per slice (2 chunks)
NCH = PADF // CH          # chunks per slice
TPC = CH // FTILE         # matmul tiles per chunk (16)
NCHUNK = SLICES * NCH     # chunks per core (8)
# Final chunk's sub-splits (frames): short end-of-pipeline chain.
SUBS = [1536, 512]

MM_DT_NP = np.float16     # matmul operand dtype (~1e-4 rel err, halves HBM)

_CACHE = {}


def _build_w() -> np.ndarray:
    w = np.zeros((128, 8), dtype=np.float32)
    for j in range(8):
        w[16 * j : 16 * j + 16, j] = 1.0 / L
    return w


def _build_nc() -> bass.Bass:
    mm_dt = mybir.dt.float16
    nc = bass.Bass()
    # Host-packed input: xz[i, c, d, 0:CH]    = low-half rows of chunk c;
    #                    xz[i, c, d, CH:2CH]  = high-half rows shifted +1.
    xz_d = nc.dram_tensor("xz", [SLICES, NCH, 128, 2 * CH], mm_dt,
                          kind="ExternalInput")
    w = nc.dram_tensor("w", [128, 8], mm_dt, kind="ExternalInput")
    # Output, partition-major: y[i, p, 128*c + 8*t_local + j]; the host
    # transposes to frame-major ((t*128+p)*8 + j), converts to fp32, and
    # trims to OUT_LEN.  fp16 output halves store traffic (values are
    # matmul fp32 results; the fp16 round adds ~5e-4 rel err, well within
    # tolerance).
    y = nc.dram_tensor("y", [SLICES, 128, NCH * TPC * 8], mybir.dt.float32,
                       kind="ExternalOutput")

    with _SingleWaitTileContext(nc) as tc:
        with (
            tc.tile_pool(name="wp", bufs=1) as wp,
            tc.tile_pool(name="xz", bufs=NCHUNK) as xzp,
            tc.tile_pool(name="zs", bufs=NCHUNK + len(SUBS) - 1) as zsp,
            tc.tile_pool(name="ob", bufs=NCHUNK) as obp,
            tc.tile_pool(name="ps", bufs=8, space="PSUM") as psp,
        ):
            # W load on the scalar (qActDynamicHW) ring; chunk loads own the
            # sync (qSPDynamicHW) ring so they complete FIFO at full rate.
            wt = wp.tile([128, 8], mm_dt)
            nc.scalar.dma_start(out=wt[:], in_=w[:])

            # All load triggers up-front on the sync ring.  Early slices load
            # as merged 2 MiB transfers (fewer ring handoffs); the final
            # chunk is split [1024, 512, 512] so the end-of-kernel chain
            # (add -> matmuls -> copy -> store) runs on 512 frames.  7 load
            # triggers + W = 8 HWDGE DMAs = the 8 DMAHW sem lanes exactly,
            # so no DMA needs a lane-reuse wait (walrus: 1 sync wait/inst).
            xt = {}          # chunk k -> (tile, column offset of chunk data)
            for k in range(NCHUNK - 1):
                t = xzp.tile([128, 2 * CH], mm_dt, tag="xc", name=f"xc{k}",
                             bufs=NCHUNK)
                i, c = k // NCH, k % NCH
                nc.sync.dma_start(out=t[:], in_=xz_d[i, c])
                xt[k] = (t, 0)
            x7 = xzp.tile([128, 2 * CH], mm_dt, tag="xc", name="x7",
                          bufs=NCHUNK)
            off = 0
            for n in SUBS:
                nc.sync.dma_start(out=x7[:, off : off + 2 * n],
                                  in_=xz_d[3, 1][:, off : off + 2 * n])
                off += 2 * n
            xt[NCHUNK - 1] = (x7, 0)

            # One full PSUM bank per chunk (8 chunks = 8 banks); full-bank
            # tiles guarantee no PE-write/ScalarE-read bank sharing.
            ps = [psp.tile([128, 512], mybir.dt.float32, tag="ps",
                           name=f"ps{k}")
                  for k in range(NCHUNK)]

            # Warmup matmul: absorbs the W-load DMA wait so no real matmul
            # ever carries two sync waits (walrus limit).  Writes garbage
            # into chunk 0's bank; chunk 0's first matmul overwrites it.
            nc.tensor.matmul(ps[0][0:8, 0:8], wt[:], wt[:],
                             start=True, stop=True)

            for k in range(NCHUNK):
                subs = [CH] if k < NCHUNK - 1 else SUBS
                base, _ = xt[k]
                xoff = xt[k][1]
                t0 = 0
                for n in subs:
                    z = zsp.tile([128, CH], mm_dt)
                    nc.vector.tensor_tensor(
                        out=z[:, 0:n],
                        in0=base[:, xoff : xoff + n],
                        in1=base[:, xoff + n : xoff + 2 * n],
                        op=mybir.AluOpType.add)
                    for t in range(n // FTILE):
                        nc.tensor.matmul(
                            ps[k][:, 8 * (t0 + t) : 8 * (t0 + t) + 8],
                            z[:, FTILE * t : FTILE * (t + 1)],
                            wt[:],
                            start=True, stop=True,
                        )
                    t0 += n // FTILE
                    xoff += 2 * n
                ob = obp.tile([128, 8 * TPC], mybir.dt.float32)
                nc.scalar.copy(ob[:], ps[k][:, 0 : 8 * TPC])
                i, c = k // NCH, k % NCH
                # SWDGE store: keeps the 8 DMASW sem lanes for stores and the
                # 8 DMAHW lanes for loads, so no DMA ever needs a lane-reuse
                # wait on top of its data wait (walrus: 1 sync wait/inst).
                nc.gpsimd.dma_start(
                    out=y[i][:, 8 * TPC * c : 8 * TPC * (c + 1)],
                    in_=ob[:],
                )
    return nc


def _get_nc():
    if "nc" not in _CACHE:
        _CACHE["nc"] = _build_nc()
    return _CACHE["nc"]


def _pack_span(xz, e16, c, col0, lo, n):
    """Pack frames [lo, lo+n) as [xl(n) | xh(n)] at chunk c, column col0."""
    hi = min(FRAMES, lo + n)
    if hi > lo:
        xz[:, c, :, col0 : col0 + hi - lo] = e16[:, 0:128, lo:hi]
    ls = max(1, lo)                       # first valid global s in the span
    le = min(lo + n - 1, FRAMES)          # last valid global s
    if le >= ls:
        xz[:, c, :, col0 + n + (ls - lo) : col0 + n + (le - lo) + 1] = \
            e16[:, 128:256, ls - 1 : le]


def _prep_inputs(est: np.ndarray) -> np.ndarray:
    """Pack [S, 256, F] fp32 into per-chunk [S, NCH, 128, 2*CH] fp16 tiles.

    Chunk layout per d-row: [xl(CH) | xh(CH)] where xl holds low rows for the
    chunk's frames (zero-padded) and xh the high rows shifted +1 frame.  The
    final chunk (c = NCH-1) of the final slice in each core's group of
    SLICES is sub-split [xlA(CHA)|xhA(CHA)|xlB(CH-CHA)|xhB(CH-CHA)] to match
    the kernel's short end-of-pipeline chain.
    """
    S = est.shape[0]
    e16 = est.astype(MM_DT_NP)
    xz = np.zeros((S, NCH, 128, 2 * CH), dtype=MM_DT_NP)
    for c in range(NCH):
        _pack_span(xz, e16, c, 0, c * CH, CH)
    # re-pack the split chunk for the last slice of each core group
    split = np.zeros((S, 128, 2 * CH), dtype=MM_DT_NP)
    sv = split[:, None]  # [S, 1, 128, 2CH] view so _pack_span's c=0 indexes it
    col = 0
    lo = (NCH - 1) * CH
    for n in SUBS:
        _pack_span(sv, e16, 0, col, lo, n)
        col += 2 * n
        lo += n
    last = np.arange(S) % SLICES == SLICES - 1
    xz[last, NCH - 1] = split[last]
    return xz


def kernel(est_source: np.ndarray, _trace: bool = False) -> np.ndarray:
    est = np.ascontiguousarray(np.asarray(est_source), dtype=np.float32)
    assert est.shape == (B, C, D2, FRAMES)
    flat = est.reshape(B * C, D2, FRAMES)
    xz = _prep_inputs(flat)
    wmat = _build_w().astype(MM_DT_NP)

    nc = _get_nc()
    in_maps = [
        {"xz": xz[SLICES * k : SLICES * (k + 1)], "w": wmat}
        for k in range(N_CORES)
    ]
    res = run_bass_kernel_spmd(nc, in_maps, core_ids=list(range(N_CORES)),
                               trace=_trace)
    _CACHE["last_results"] = res
    # y[i, p, 128c + 8t_local + j] -> out[i, (128(16c+t_local)+p)*8 + j]
    ys = [res.results[k]["y"] for k in range(N_CORES)]
    y = np.concatenate(ys, axis=0).astype(np.float32)    # [32, 128, 256]
    y = y.reshape(B * C, 128, NCH * TPC, 8)              # [32, p, t, j]
    y = y.transpose(0, 2, 1, 3).reshape(B * C, PADF * 8)  # [(t p j)]
    return np.ascontiguousarray(y[:, :OUT_LEN]).reshape(B, C, OUT_LEN)
